# revision 1
# baseline (speedup 1.0000x reference)
"""TRN2 Bass/Tile kernel for nn_AttentionTemporalEncoder (B=32, H=1024, T=512, 16 heads).

Strategy: data-parallel over batch - 4 batches per NeuronCore on 8 cores,
weights replicated to every core.  Per batch, entirely on-chip:

  Qt = (Wq @ X + bq)/8 and Kt = Wk @ X + bk, hidden-major [H, T]; Q is stored
  as two zero-padded variants (even heads in partition rows 0-63, odd heads in
  64-127) so every scores matmul contracts over the full K=128 rows - the
  other head's rows multiply zeros - keeping all PE matmuls the same full-row
  shape so LDWEIGHTS pipelines.
  V = X.T @ Wv.T, T-major, with a ones-column per head so the attention
  matmul also emits the softmax denominators for free.
  Scores are computed transposed [Tk, Tq]: the key-padding mask becomes a
  per-partition bias (-1e4) folded into the Exp activation (exp -> exact 0);
  no max-subtraction needed (|scores| <~ 3).
  Oraw_h = (V_h.T @ exp(scores^T)) * (1/denominators)  (fast DVE reciprocal,
  gpsimd partition-broadcast).
  y^T = Wo @ Oraw emitted transposed so the final max over T is a free-dim
  reduce_max.  Host folds 1/sqrt(64) into Wq/bq and adds Wo@bv + bo after the
  gather (max over T commutes with per-channel constants).

The emission software-pipelines everything: projections of batch b+1 and the
output projection of batch b-1 are interleaved into batch b's attention so the
PE never waits on the exp/normalize dependency chains.  Compute dtype fp16
(fp32 PSUM accumulation); measured rel err vs fp32 reference ~3.4e-4.
Measured HW time: 378.5-386us across six full-clock runs (median ~380us);
the spread is device scheduler/DMA variance at identical PE-active time.
"""

import os
import sys

import numpy as np

for _p in ("/opt/trn_rl_repo", "/opt/pypackages"):
    if os.path.isdir(_p) and _p not in sys.path:
        sys.path.append(_p)

import concourse.tile as tile
from concourse import bacc, mybir

F32 = mybir.dt.float32
DT = mybir.dt.float16
NP_DT = np.float16

B, H, T = 32, 1024, 512
NH, HD = 16, 64
N_CORES = 8
NB = B // N_CORES  # batches per core
OC = H // 128      # hidden chunks
TC = T // 128      # token chunks


def _build_nc():
    dt = DT
    nb = NB
    nc = bacc.Bacc("TRN2", target_bir_lowering=False, debug=False, num_devices=N_CORES)

    x_d = nc.dram_tensor("x", [nb, OC, 128, T], dt, kind="ExternalInput")
    wqT_d = nc.dram_tensor("wqT", [OC, 128, H], dt, kind="ExternalInput")
    wkT_d = nc.dram_tensor("wkT", [OC, 128, H], dt, kind="ExternalInput")
    wvT_d = nc.dram_tensor("wvT", [OC, 128, H], dt, kind="ExternalInput")
    woT_d = nc.dram_tensor("woT", [OC, 128, H], dt, kind="ExternalInput")
    bq_d = nc.dram_tensor("bq", [OC, 128], F32, kind="ExternalInput")
    bk_d = nc.dram_tensor("bk", [OC, 128], F32, kind="ExternalInput")
    mb_d = nc.dram_tensor("maskbias", [nb, TC, 128], F32, kind="ExternalInput")
    y_d = nc.dram_tensor("y", [nb, OC, 128], F32, kind="ExternalOutput")

    from contextlib import ExitStack

    with tile.TileContext(nc) as tc, ExitStack() as ctx:
        consts = ctx.enter_context(tc.tile_pool(name="consts", bufs=1))
        xpool = ctx.enter_context(tc.tile_pool(name="xpool", bufs=4))
        qkv = ctx.enter_context(tc.tile_pool(name="qkv", bufs=2))
        attnp = ctx.enter_context(tc.tile_pool(name="attnp", bufs=4))
        acts = ctx.enter_context(tc.tile_pool(name="acts", bufs=1))
        smallp = ctx.enter_context(tc.tile_pool(name="smallp", bufs=4))
        resp = ctx.enter_context(tc.tile_pool(name="resp", bufs=2))
        ps_proj = ctx.enter_context(tc.tile_pool(name="ps_proj", bufs=4, space="PSUM"))
        ps_sc = ctx.enter_context(tc.tile_pool(name="ps_sc", bufs=2, space="PSUM"))
        ps_ao = ctx.enter_context(tc.tile_pool(name="ps_ao", bufs=2, space="PSUM"))

        # ---- persistent constants (x0 + small consts + wq first so compute starts early) ----
        bq_sb = consts.tile([128, OC], F32, tag="bq")
        bk_sb = consts.tile([128, OC], F32, tag="bk")
        mb_sb = consts.tile([128, nb * TC], F32, tag="mb")
        x_tiles = [xpool.tile([128, OC, T], dt, tag="x", name=f"x{b}") for b in range(nb)]
        for ic in range(OC):
            nc.sync.dma_start(out=x_tiles[0][:, ic, :], in_=x_d.ap()[0, ic])
        nc.sync.dma_start(out=bq_sb[:, :], in_=bq_d.ap().rearrange("c p -> p c"))
        nc.sync.dma_start(out=bk_sb[:, :], in_=bk_d.ap().rearrange("c p -> p c"))
        nc.sync.dma_start(out=mb_sb[:, :], in_=mb_d.ap().rearrange("b c p -> p (b c)"))
        wq_sb = consts.tile([128, OC, H], dt, tag="wq")
        wk_sb = consts.tile([128, OC, H], dt, tag="wk")
        wv_sb = consts.tile([128, OC, H], dt, tag="wv")
        wo_sb = consts.tile([128, OC, H], dt, tag="wo")
        for ic in range(OC):
            nc.scalar.dma_start(out=wq_sb[:, ic, :], in_=wqT_d.ap()[ic])
        for ic in range(OC):
            nc.sync.dma_start(out=wk_sb[:, ic, :], in_=wkT_d.ap()[ic])
        for ic in range(OC):
            nc.scalar.dma_start(out=wv_sb[:, ic, :], in_=wvT_d.ap()[ic])
        for ic in range(OC):
            nc.scalar.dma_start(out=wo_sb[:, ic, :], in_=woT_d.ap()[ic])
        for b in range(1, nb):
            for ic in range(OC):
                nc.sync.dma_start(out=x_tiles[b][:, ic, :], in_=x_d.ap()[b, ic])

        outraw_tiles = {}

        # persistent activation tiles, reused in place across batches (WAR
        # deps order next-batch writes after this batch's reads)
        qte_sb = acts.tile([128, OC, T], dt, tag="qte")
        qto_sb = acts.tile([128, OC, T], dt, tag="qto")
        kt_sb = acts.tile([128, OC, T], dt, tag="kt")
        v_sb = acts.tile([128, TC, NH, HD + 1], dt, tag="v")
        nc.vector.memset(v_sb[:, :, :, HD:HD + 1], 1.0)
        nc.vector.memset(qte_sb[64:128, :, :], 0.0)
        nc.vector.memset(qto_sb[0:64, :, :], 0.0)

        def emit_yproj_chunk(res_sb, outraw_sb, oc):
            ps = ps_proj.tile([128, T], F32, tag="proj", name="ps_y")
            for cc in range(OC):
                nc.tensor.matmul(
                    ps[:, :],
                    lhsT=wo_sb[:, cc, 128 * oc:128 * (oc + 1)],
                    rhs=outraw_sb[:, cc, :],
                    start=(cc == 0),
                    stop=(cc == OC - 1),
                )
            nc.vector.reduce_max(res_sb[:, oc:oc + 1], ps[:, :], axis=mybir.AxisListType.X)

        def emit_yproj(b, outraw_sb):
            # ---- output projection (transposed) + max over T ----
            res_sb = resp.tile([128, OC], F32, tag="res", name=f"res{b}")
            for oc in range(OC):
                emit_yproj_chunk(res_sb, outraw_sb, oc)
            nc.gpsimd.dma_start(out=y_d.ap()[b].rearrange("c p -> p c"), in_=res_sb[:, :])

        for b in range(nb):
            x_sb = x_tiles[b]

            # ---- projections, emitted interleaved with attention below ----
            # Q lives as two zero-padded variants (even heads in rows 0-63,
            # odd heads in rows 64-127) so scores matmuls use full K=128:
            # the other head's rows multiply zeros.  Every matmul is then a
            # full-row op and the PE pipelines every LDWEIGHTS.
            def emit_qk(oc, x_sb=x_sb):  # default: this batch's x
                ps = ps_proj.tile([128, T], F32, tag="proj", name="ps_qk")
                for ic in range(OC):
                    nc.tensor.matmul(
                        ps[:, :],
                        lhsT=wq_sb[:, ic, 128 * oc:128 * (oc + 1)],
                        rhs=x_sb[:, ic, :],
                        start=(ic == 0),
                        stop=(ic == OC - 1),
                    )
                nc.vector.tensor_scalar_add(qte_sb[0:64, oc, :], ps[0:64, :], bq_sb[0:64, oc:oc + 1])
                nc.vector.tensor_scalar_add(qto_sb[64:128, oc, :], ps[64:128, :], bq_sb[64:128, oc:oc + 1])
                ps = ps_proj.tile([128, T], F32, tag="proj", name="ps_qk2")
                for ic in range(OC):
                    nc.tensor.matmul(
                        ps[:, :],
                        lhsT=wk_sb[:, ic, 128 * oc:128 * (oc + 1)],
                        rhs=x_sb[:, ic, :],
                        start=(ic == 0),
                        stop=(ic == OC - 1),
                    )
                nc.vector.tensor_scalar_add(kt_sb[:, oc, :], ps[:, :], bk_sb[:, oc:oc + 1])

            def emit_vhalf(ch, x_sb=x_sb):
                for tcc in range(TC):
                    ps = ps_proj.tile([128, T], F32, tag="proj", name="ps_v")
                    for ic in range(OC):
                        nc.tensor.matmul(
                            ps[:, :],
                            lhsT=x_sb[:, ic, 128 * tcc:128 * (tcc + 1)],
                            rhs=wv_sb[:, ic, 512 * ch:512 * (ch + 1)],
                            start=(ic == 0),
                            stop=(ic == OC - 1),
                        )
                    nc.vector.tensor_copy(
                        out=v_sb[:, tcc, 8 * ch:8 * (ch + 1), 0:HD],
                        in_=ps[:, :].rearrange("p (h d) -> p h d", h=8),
                    )

            # ---- attention, two heads at a time, scores one pair ahead of attnV ----
            outraw_sb = qkv.tile([128, OC, T], dt, tag="outraw", name=f"outraw{b}")
            outraw_tiles[b] = outraw_sb

            def emit_scores(hp):
                heads = (2 * hp, 2 * hp + 1)
                attns = (attnp.tile([128, TC, T], dt, tag="attn0", name="attn0"),
                         attnp.tile([128, TC, T], dt, tag="attn1", name="attn1"))
                for tcc in range(TC):
                    for h, attn_sb in zip(heads, attns):
                        hc = h // 2
                        q_sb = qte_sb if h % 2 == 0 else qto_sb
                        ps_s = ps_sc.tile([128, T], F32, tag="sc")
                        nc.tensor.matmul(
                            ps_s[:, :],
                            lhsT=kt_sb[:, hc, 128 * tcc:128 * (tcc + 1)],
                            rhs=q_sb[:, hc, :],
                            start=True,
                            stop=True,
                        )
                        nc.scalar.activation(
                            attn_sb[:, tcc, :],
                            ps_s[:, :],
                            mybir.ActivationFunctionType.Exp,
                            bias=mb_sb[:, b * TC + tcc:b * TC + tcc + 1],
                            scale=1.0,
                        )
                return attns

            def emit_attnv(hp, attns):
                for h, attn_sb in zip((2 * hp, 2 * hp + 1), attns):
                    hc, ho = h // 2, 64 * (h % 2)
                    ps_o = ps_ao.tile([HD + 1, T], F32, tag="ao")
                    for tcc in range(TC):
                        nc.tensor.matmul(
                            ps_o[:, :],
                            lhsT=v_sb[:, tcc, h, :],
                            rhs=attn_sb[:, tcc, :],
                            start=(tcc == 0),
                            stop=(tcc == TC - 1),
                        )
                    sums1 = smallp.tile([1, T], F32, tag="sums1")
                    nc.vector.tensor_copy(out=sums1[:, :], in_=ps_o[HD:HD + 1, :])
                    recip1 = smallp.tile([1, T], F32, tag="recip1")
                    nc.vector.reciprocal_approx_fast(recip1[:, :], sums1[:, :])
                    recip64 = smallp.tile([64, T], F32, tag="recip64")
                    nc.gpsimd.partition_broadcast(recip64[:, :], recip1[:, :])
                    nc.vector.tensor_mul(outraw_sb[ho:ho + 64, hc, :], ps_o[0:HD, :], recip64[:, :])

            if b > 0:
                res_prev = resp.tile([128, OC], F32, tag="res", name=f"res{b - 1}")

            def yp(j):
                if b > 0:
                    emit_yproj_chunk(res_prev, outraw_tiles[b - 1], j)

            # interleave projection chunks with attention pairs: scores(p) only
            # needs QK chunk p (heads 2p, 2p+1), attnV pair p needs V half p//4
            if b == 0:
                emit_qk(0)
                emit_qk(1)
                emit_vhalf(0)
            prev = emit_scores(0)
            for hp in range(1, NH // 2):
                if hp + 1 < OC:
                    emit_qk(hp + 1)
                if hp == 4:
                    emit_vhalf(1)
                cur = emit_scores(hp)
                emit_attnv(hp - 1, prev)
                yp(hp - 1)
                if b + 1 < nb:
                    if hp == 4:
                        emit_qk(0, x_tiles[b + 1])
                    if hp == 5:
                        emit_qk(1, x_tiles[b + 1])
                    if hp == 6:
                        emit_vhalf(0, x_tiles[b + 1])
                prev = cur
            emit_attnv(NH // 2 - 1, prev)
            yp(NH // 2 - 1)
            if b > 0:
                nc.gpsimd.dma_start(out=y_d.ap()[b - 1].rearrange("c p -> p c"), in_=res_prev[:, :])

        emit_yproj(nb - 1, outraw_tiles[nb - 1])

    nc.compile()
    return nc


_NC_CACHE = None


def _get_nc():
    global _NC_CACHE
    if _NC_CACHE is None:
        _NC_CACHE = _build_nc()
    return _NC_CACHE


def kernel(x, mask, Wq, bq, Wk, bk, Wv, bv, Wo, bo):
    x = np.asarray(x, dtype=np.float32)
    mask = np.asarray(mask)
    Wq, bq, Wk, bk, Wv, bv, Wo, bo = (
        np.asarray(a, dtype=np.float32) for a in (Wq, bq, Wk, bk, Wv, bv, Wo, bo)
    )
    scale = np.float32(1.0 / np.sqrt(np.float32(HD)))

    wqT = np.ascontiguousarray((Wq.T * scale).reshape(OC, 128, H).astype(NP_DT))
    wkT = np.ascontiguousarray(Wk.T.reshape(OC, 128, H).astype(NP_DT))
    wvT = np.ascontiguousarray(Wv.T.reshape(OC, 128, H).astype(NP_DT))
    woT = np.ascontiguousarray(Wo.T.reshape(OC, 128, H).astype(NP_DT))
    bq_s = np.ascontiguousarray((bq * scale).reshape(OC, 128).astype(np.float32))
    bk_s = np.ascontiguousarray(bk.reshape(OC, 128).astype(np.float32))
    maskbias = np.where(mask == 0, np.float32(-10000.0), np.float32(0.0)).astype(np.float32)

    in_maps = []
    for c in range(N_CORES):
        sl = slice(c * NB, (c + 1) * NB)
        in_maps.append({
            "x": np.ascontiguousarray(x[sl].reshape(NB, OC, 128, T).astype(NP_DT)),
            "wqT": wqT, "wkT": wkT, "wvT": wvT, "woT": woT,
            "bq": bq_s, "bk": bk_s,
            "maskbias": np.ascontiguousarray(maskbias[sl].reshape(NB, TC, 128)),
        })

    from concourse.bass_utils import run_bass_kernel_spmd

    nc = _get_nc()
    res = run_bass_kernel_spmd(nc, in_maps, core_ids=list(range(N_CORES)))
    y = np.concatenate(
        [res.results[i]["y"].reshape(NB, H) for i in range(N_CORES)], axis=0
    )
    # max over T commutes with the per-channel constant Wo @ bv + bo
    bo2 = Wo @ bv + bo
    return (y + bo2[None, :]).astype(np.float32)



# revision 2
# speedup vs baseline: 1.1077x; 1.1077x over previous
"""TRN2 Bass/Tile kernel for nn_AttentionTemporalEncoder (B=32, H=1024, T=512, 16 heads).

Strategy: data-parallel over batch - 4 batches per NeuronCore on 8 cores,
weights replicated to every core.  Per batch, entirely on-chip:

  Key compaction: the key-padding mask zeroes ~half the keys exactly
  (masked scores -> exp(-1e4) = 0), so the host gathers only the unmasked
  key columns of x per batch (TK_b ~ 256 of 512) and pads to the global
  max TKP; K-proj, V-proj, scores and attnV then run over ceil(TKP/128)=3
  key chunks instead of 4 - PE matmul time on TRN2 is (free-dim columns)
  x cycles_per_row, independent of partial partition dims.  Pad columns
  are zero and carry a -1e4 exp-bias (per-batch data, keeping the program
  SPMD-identical across cores).

  Qt = (Wq @ X + bq)/8 hidden-major [H, T] over all 512 queries; Q is
  stored as two zero-padded variants (even heads in partition rows 0-63,
  odd heads in 64-127) so every scores matmul contracts over the full
  K=128 rows.  Kt = Wk @ Xk + bk is [H, TKP].
  V = Xk.T @ Wv.T, key-major, with a ones-column per head so the attention
  matmul also emits the softmax denominators for free.
  Scores are computed transposed [TKP, Tq] in KC chunks; the pad-key bias
  (-1e4) is folded into the Exp activation (exp -> exact 0); no
  max-subtraction needed (|scores| <~ 3).
  Oraw_h = (V_h.T @ exp(scores^T)) * (1/denominators)  (fast DVE reciprocal,
  gpsimd partition-broadcast).
  y^T = Wo @ Oraw emitted transposed so the final max over T is a free-dim
  reduce_max.  Host folds 1/sqrt(64) into Wq/bq and adds Wo@bv + bo after
  the gather (max over T commutes with per-channel constants).

The emission software-pipelines everything: projections of batch b+1 and the
output projection of batch b-1 are interleaved into batch b's attention so the
PE never waits on the exp/normalize dependency chains.  Compute dtype fp16
(fp32 PSUM accumulation).
"""

import os
import sys

import numpy as np

for _p in ("/opt/trn_rl_repo", "/opt/pypackages"):
    if os.path.isdir(_p) and _p not in sys.path:
        sys.path.append(_p)

import concourse.tile as tile
from concourse import bacc, mybir

F32 = mybir.dt.float32
DT = mybir.dt.float16
NP_DT = np.float16

B, H, T = 32, 1024, 512
NH, HD = 16, 64
N_CORES = 8
NB = B // N_CORES  # batches per core
OC = H // 128      # hidden chunks
TC = T // 128      # token chunks (queries)


def _build_nc(tkp):
    dt = DT
    nb = NB
    kc_n = (tkp + 127) // 128          # key chunks
    mks = [min(128, tkp - 128 * kc) for kc in range(kc_n)]  # rows per chunk
    nc = bacc.Bacc("TRN2", target_bir_lowering=False, debug=False, num_devices=N_CORES)

    x_d = nc.dram_tensor("x", [nb, OC, 128, T], dt, kind="ExternalInput")
    xk_d = nc.dram_tensor("xk", [nb, OC, 128, tkp], dt, kind="ExternalInput")
    wqT_d = nc.dram_tensor("wqT", [OC, 128, H], dt, kind="ExternalInput")
    wkT_d = nc.dram_tensor("wkT", [OC, 128, H], dt, kind="ExternalInput")
    wvT_d = nc.dram_tensor("wvT", [OC, 128, H], dt, kind="ExternalInput")
    woT_d = nc.dram_tensor("woT", [OC, 128, H], dt, kind="ExternalInput")
    bq_d = nc.dram_tensor("bq", [OC, 128], F32, kind="ExternalInput")
    bk_d = nc.dram_tensor("bk", [OC, 128], F32, kind="ExternalInput")
    mb_d = nc.dram_tensor("maskbias", [nb, kc_n, 128], F32, kind="ExternalInput")
    y_d = nc.dram_tensor("y", [nb, OC, 128], F32, kind="ExternalOutput")

    from contextlib import ExitStack

    with tile.TileContext(nc) as tc, ExitStack() as ctx:
        consts = ctx.enter_context(tc.tile_pool(name="consts", bufs=1))
        xpool = ctx.enter_context(tc.tile_pool(name="xpool", bufs=4))
        qkv = ctx.enter_context(tc.tile_pool(name="qkv", bufs=2))
        attnp = ctx.enter_context(tc.tile_pool(name="attnp", bufs=4))
        acts = ctx.enter_context(tc.tile_pool(name="acts", bufs=1))
        smallp = ctx.enter_context(tc.tile_pool(name="smallp", bufs=4))
        resp = ctx.enter_context(tc.tile_pool(name="resp", bufs=2))
        ps_proj = ctx.enter_context(tc.tile_pool(name="ps_proj", bufs=4, space="PSUM"))
        ps_sc = ctx.enter_context(tc.tile_pool(name="ps_sc", bufs=2, space="PSUM"))
        ps_ao = ctx.enter_context(tc.tile_pool(name="ps_ao", bufs=2, space="PSUM"))

        # ---- persistent constants (x0 + small consts + wq first so compute starts early) ----
        bq_sb = consts.tile([128, OC], F32, tag="bq")
        bk_sb = consts.tile([128, OC], F32, tag="bk")
        mb_sb = consts.tile([128, nb * kc_n], F32, tag="mb")
        x_tiles = [xpool.tile([128, OC, T], dt, tag="x", name=f"x{b}") for b in range(nb)]
        xk_tiles = [xpool.tile([128, OC, tkp], dt, tag="xk", name=f"xk{b}") for b in range(nb)]
        for ic in range(OC):
            nc.sync.dma_start(out=x_tiles[0][:, ic, :], in_=x_d.ap()[0, ic])
        for ic in range(OC):
            nc.sync.dma_start(out=xk_tiles[0][:, ic, :], in_=xk_d.ap()[0, ic])
        nc.sync.dma_start(out=bq_sb[:, :], in_=bq_d.ap().rearrange("c p -> p c"))
        nc.sync.dma_start(out=bk_sb[:, :], in_=bk_d.ap().rearrange("c p -> p c"))
        nc.sync.dma_start(out=mb_sb[:, :], in_=mb_d.ap().rearrange("b c p -> p (b c)"))
        wq_sb = consts.tile([128, OC, H], dt, tag="wq")
        wk_sb = consts.tile([128, OC, H], dt, tag="wk")
        wv_sb = consts.tile([128, OC, H], dt, tag="wv")
        wo_sb = consts.tile([128, OC, H], dt, tag="wo")
        for ic in range(OC):
            nc.scalar.dma_start(out=wq_sb[:, ic, :], in_=wqT_d.ap()[ic])
        for ic in range(OC):
            nc.sync.dma_start(out=wk_sb[:, ic, :], in_=wkT_d.ap()[ic])
        for ic in range(OC):
            nc.scalar.dma_start(out=wv_sb[:, ic, :], in_=wvT_d.ap()[ic])
        for ic in range(OC):
            nc.scalar.dma_start(out=wo_sb[:, ic, :], in_=woT_d.ap()[ic])
        for b in range(1, nb):
            for ic in range(OC):
                nc.sync.dma_start(out=x_tiles[b][:, ic, :], in_=x_d.ap()[b, ic])
            for ic in range(OC):
                nc.sync.dma_start(out=xk_tiles[b][:, ic, :], in_=xk_d.ap()[b, ic])

        outraw_tiles = {}

        # persistent activation tiles, reused in place across batches (WAR
        # deps order next-batch writes after this batch's reads)
        qte_sb = acts.tile([128, OC, T], dt, tag="qte")
        qto_sb = acts.tile([128, OC, T], dt, tag="qto")
        kt_sb = acts.tile([128, OC, tkp], dt, tag="kt")
        v_sb = acts.tile([128, kc_n, NH, HD + 1], dt, tag="v")
        nc.vector.memset(v_sb[:, :, :, HD:HD + 1], 1.0)
        nc.vector.memset(qte_sb[64:128, :, :], 0.0)
        nc.vector.memset(qto_sb[0:64, :, :], 0.0)

        def emit_yproj_chunk(res_sb, outraw_sb, oc):
            ps = ps_proj.tile([128, T], F32, tag="proj", name="ps_y")
            for cc in range(OC):
                nc.tensor.matmul(
                    ps[:, :],
                    lhsT=wo_sb[:, cc, 128 * oc:128 * (oc + 1)],
                    rhs=outraw_sb[:, cc, :],
                    start=(cc == 0),
                    stop=(cc == OC - 1),
                )
            nc.vector.reduce_max(res_sb[:, oc:oc + 1], ps[:, :], axis=mybir.AxisListType.X)

        def emit_yproj(b, outraw_sb):
            # ---- output projection (transposed) + max over T ----
            res_sb = resp.tile([128, OC], F32, tag="res", name=f"res{b}")
            for oc in range(OC):
                emit_yproj_chunk(res_sb, outraw_sb, oc)
            nc.gpsimd.dma_start(out=y_d.ap()[b].rearrange("c p -> p c"), in_=res_sb[:, :])

        for b in range(nb):
            x_sb = x_tiles[b]
            xk_sb = xk_tiles[b]

            # ---- projections, emitted interleaved with attention below ----
            # Q lives as two zero-padded variants (even heads in rows 0-63,
            # odd heads in rows 64-127) so scores matmuls use full K=128:
            # the other head's rows multiply zeros.  Every matmul is then a
            # full-row op and the PE pipelines every LDWEIGHTS.
            def emit_qk(oc, x_sb=x_sb, xk_sb=xk_sb):  # default: this batch's x
                ps = ps_proj.tile([128, T], F32, tag="proj", name="ps_qk")
                for ic in range(OC):
                    nc.tensor.matmul(
                        ps[:, :],
                        lhsT=wq_sb[:, ic, 128 * oc:128 * (oc + 1)],
                        rhs=x_sb[:, ic, :],
                        start=(ic == 0),
                        stop=(ic == OC - 1),
                    )
                nc.vector.tensor_scalar_add(qte_sb[0:64, oc, :], ps[0:64, :], bq_sb[0:64, oc:oc + 1])
                nc.vector.tensor_scalar_add(qto_sb[64:128, oc, :], ps[64:128, :], bq_sb[64:128, oc:oc + 1])
                ps = ps_proj.tile([128, T], F32, tag="proj", name="ps_qk2")
                for ic in range(OC):
                    nc.tensor.matmul(
                        ps[:, 0:tkp],
                        lhsT=wk_sb[:, ic, 128 * oc:128 * (oc + 1)],
                        rhs=xk_sb[:, ic, :],
                        start=(ic == 0),
                        stop=(ic == OC - 1),
                    )
                nc.vector.tensor_scalar_add(kt_sb[:, oc, :], ps[:, 0:tkp], bk_sb[:, oc:oc + 1])

            def emit_vhalf(ch, xk_sb=xk_sb):
                for kcc in range(kc_n):
                    mk = mks[kcc]
                    ps = ps_proj.tile([128, T], F32, tag="proj", name="ps_v")
                    for ic in range(OC):
                        nc.tensor.matmul(
                            ps[0:mk, :],
                            lhsT=xk_sb[:, ic, 128 * kcc:128 * kcc + mk],
                            rhs=wv_sb[:, ic, 512 * ch:512 * (ch + 1)],
                            start=(ic == 0),
                            stop=(ic == OC - 1),
                        )
                    nc.vector.tensor_copy(
                        out=v_sb[0:mk, kcc, 8 * ch:8 * (ch + 1), 0:HD],
                        in_=ps[0:mk, :].rearrange("p (h d) -> p h d", h=8),
                    )

            # ---- attention, two heads at a time, scores one pair ahead of attnV ----
            outraw_sb = qkv.tile([128, OC, T], dt, tag="outraw", name=f"outraw{b}")
            outraw_tiles[b] = outraw_sb

            def emit_scores(hp):
                heads = (2 * hp, 2 * hp + 1)
                attns = (attnp.tile([128, kc_n, T], dt, tag="attn0", name="attn0"),
                         attnp.tile([128, kc_n, T], dt, tag="attn1", name="attn1"))
                for kcc in range(kc_n):
                    mk = mks[kcc]
                    for h, attn_sb in zip(heads, attns):
                        hc = h // 2
                        q_sb = qte_sb if h % 2 == 0 else qto_sb
                        ps_s = ps_sc.tile([128, T], F32, tag="sc")
                        nc.tensor.matmul(
                            ps_s[0:mk, :],
                            lhsT=kt_sb[:, hc, 128 * kcc:128 * kcc + mk],
                            rhs=q_sb[:, hc, :],
                            start=True,
                            stop=True,
                        )
                        nc.scalar.activation(
                            attn_sb[0:mk, kcc, :],
                            ps_s[0:mk, :],
                            mybir.ActivationFunctionType.Exp,
                            bias=mb_sb[0:mk, b * kc_n + kcc:b * kc_n + kcc + 1],
                            scale=1.0,
                        )
                return attns

            def emit_attnv(hp, attns):
                for h, attn_sb in zip((2 * hp, 2 * hp + 1), attns):
                    hc, ho = h // 2, 64 * (h % 2)
                    ps_o = ps_ao.tile([HD + 1, T], F32, tag="ao")
                    for kcc in range(kc_n):
                        mk = mks[kcc]
                        nc.tensor.matmul(
                            ps_o[:, :],
                            lhsT=v_sb[0:mk, kcc, h, :],
                            rhs=attn_sb[0:mk, kcc, :],
                            start=(kcc == 0),
                            stop=(kcc == kc_n - 1),
                        )
                    sums1 = smallp.tile([1, T], F32, tag="sums1")
                    nc.vector.tensor_copy(out=sums1[:, :], in_=ps_o[HD:HD + 1, :])
                    recip1 = smallp.tile([1, T], F32, tag="recip1")
                    nc.vector.reciprocal_approx_fast(recip1[:, :], sums1[:, :])
                    recip64 = smallp.tile([64, T], F32, tag="recip64")
                    nc.gpsimd.partition_broadcast(recip64[:, :], recip1[:, :])
                    nc.vector.tensor_mul(outraw_sb[ho:ho + 64, hc, :], ps_o[0:HD, :], recip64[:, :])

            if b > 0:
                res_prev = resp.tile([128, OC], F32, tag="res", name=f"res{b - 1}")

            def yp(j):
                if b > 0:
                    emit_yproj_chunk(res_prev, outraw_tiles[b - 1], j)

            # interleave projection chunks with attention pairs: scores(p) only
            # needs QK chunk p (heads 2p, 2p+1), attnV pair p needs V half p//4
            if b == 0:
                emit_qk(0)
                emit_qk(1)
                emit_vhalf(0)
            prev = emit_scores(0)
            for hp in range(1, NH // 2):
                if hp + 1 < OC:
                    emit_qk(hp + 1)
                if hp == 4:
                    emit_vhalf(1)
                cur = emit_scores(hp)
                emit_attnv(hp - 1, prev)
                yp(hp - 1)
                if b + 1 < nb:
                    if hp == 4:
                        emit_qk(0, x_tiles[b + 1], xk_tiles[b + 1])
                    if hp == 5:
                        emit_qk(1, x_tiles[b + 1], xk_tiles[b + 1])
                    if hp == 6:
                        emit_vhalf(0, xk_tiles[b + 1])
                prev = cur
            emit_attnv(NH // 2 - 1, prev)
            yp(NH // 2 - 1)
            if b > 0:
                nc.gpsimd.dma_start(out=y_d.ap()[b - 1].rearrange("c p -> p c"), in_=res_prev[:, :])

        emit_yproj(nb - 1, outraw_tiles[nb - 1])

    nc.compile()
    return nc


_NC_CACHE = {}


def _get_nc(tkp):
    if tkp not in _NC_CACHE:
        _NC_CACHE[tkp] = _build_nc(tkp)
    return _NC_CACHE[tkp]


def _prep(x, mask, Wq, bq, Wk, bk, Wv, bv, Wo, bo):
    """Host-side prep: fold scales, gather unmasked key columns, shard."""
    x = np.asarray(x, dtype=np.float32)
    mask = np.asarray(mask)
    Wq, bq, Wk, bk, Wv, bv, Wo, bo = (
        np.asarray(a, dtype=np.float32) for a in (Wq, bq, Wk, bk, Wv, bv, Wo, bo)
    )
    scale = np.float32(1.0 / np.sqrt(np.float32(HD)))

    wqT = np.ascontiguousarray((Wq.T * scale).reshape(OC, 128, H).astype(NP_DT))
    wkT = np.ascontiguousarray(Wk.T.reshape(OC, 128, H).astype(NP_DT))
    wvT = np.ascontiguousarray(Wv.T.reshape(OC, 128, H).astype(NP_DT))
    woT = np.ascontiguousarray(Wo.T.reshape(OC, 128, H).astype(NP_DT))
    bq_s = np.ascontiguousarray((bq * scale).reshape(OC, 128).astype(np.float32))
    bk_s = np.ascontiguousarray(bk.reshape(OC, 128).astype(np.float32))

    idx = [np.nonzero(mask[b] != 0)[0] for b in range(B)]
    tks = [len(i) for i in idx]
    tkp = max(tks)
    kc_n = (tkp + 127) // 128

    x16 = x.astype(NP_DT)
    xk = np.zeros((B, H, tkp), dtype=NP_DT)
    for b in range(B):
        xk[b, :, : tks[b]] = x16[b][:, idx[b]]
    maskbias = np.zeros((B, kc_n * 128), dtype=np.float32)
    for b in range(B):
        maskbias[b, tks[b]:] = np.float32(-10000.0)

    in_maps = []
    for c in range(N_CORES):
        sl = slice(c * NB, (c + 1) * NB)
        in_maps.append({
            "x": np.ascontiguousarray(x16[sl].reshape(NB, OC, 128, T)),
            "xk": np.ascontiguousarray(xk[sl].reshape(NB, OC, 128, tkp)),
            "wqT": wqT, "wkT": wkT, "wvT": wvT, "woT": woT,
            "bq": bq_s, "bk": bk_s,
            "maskbias": np.ascontiguousarray(maskbias[sl].reshape(NB, kc_n, 128)),
        })
    return in_maps, tkp


def kernel(x, mask, Wq, bq, Wk, bk, Wv, bv, Wo, bo):
    in_maps, tkp = _prep(x, mask, Wq, bq, Wk, bk, Wv, bv, Wo, bo)

    from concourse.bass_utils import run_bass_kernel_spmd

    nc = _get_nc(tkp)
    res = run_bass_kernel_spmd(nc, in_maps, core_ids=list(range(N_CORES)))
    y = np.concatenate(
        [res.results[i]["y"].reshape(NB, H) for i in range(N_CORES)], axis=0
    )
    # max over T commutes with the per-channel constant Wo @ bv + bo
    Wo = np.asarray(Wo, dtype=np.float32)
    bv = np.asarray(bv, dtype=np.float32)
    bo = np.asarray(bo, dtype=np.float32)
    bo2 = Wo @ bv + bo
    return (y + bo2[None, :]).astype(np.float32)
